# revision 33
# baseline (speedup 1.0000x reference)
"""Trainium2 Bass kernel: batched int8 dequant-BMM.

out[b] = (x[b].f32 - a_zp) @ (y[b].f32 - b_zp) * alpha
  x: [96, 1024, 64] int8, y: [96, 64, 1024] int8 -> out: [96, 1024, 1024] f32

Sharding: batch dim 96 -> 12 per core across 8 cores (pure data parallel).

The store stream saturates HBM (~358 GB/s/core), which trips the chip's
activity throttle: the PE is clock-gated to K=4/8 (~1.2 col/ns) for the
whole main phase, making PE column count the binding roofline. Hence:
  - x is pre-transposed on the HOST (numpy) so the kernel needs no PE
    transposes (saves 6144 PE columns/core + the DVE copy-backs + the
    identity constant; lhsT reads the dequanted x_T with a stride-8
    free-dim AP instead).
  - output is stored as fp16 and upcast on the host: the PSUM value is
    the exact integer result; alpha-scale + fp16 round gives max rel
    err 2^-11 ~= 4.9e-4, 40x inside the 2e-2 gate, and halves the
    store traffic.

Per-core pipeline (batch pair at a time; even batch on PE rows 0-63,
odd batch on rows 64-127 so the K=64 contractions row-tile the PE):
  DMA x_T,y int8 -> ACT dequant to bf16 (exact: integers < 256)
  -> PE matmul bf16 -> fp32 PSUM (exact: sums < 2^24)
  -> ACT/DVE copy PSUM->SBUF fused with *alpha, cast fp16 -> DMA out.

All DMA rides the two HWDGE rings. dma_start costs ~630ns on the
issuing engine, so the sync ring carries everything (loads interleaved
between store groups) and ACT issues none; the scalar ring is unused.
HAM warmup (~3.4us of dummy matmuls on memset data while PE waits on
the first loads) buys K=8/8 for the first ~2 pairs of real matmuls.
"""

import numpy as np

B, S, D = 96, 1024, 64
N_CORES = 8
BPC = B // N_CORES  # batches per core = 12
NPAIRS = BPC // 2

_cache = {}


def _build(az: float, bz: float, al: float):
    key = (az, bz, al)
    if key in _cache:
        return _cache[key]

    from contextlib import ExitStack

    import concourse.mybir as mybir
    import concourse.tile as tile
    from concourse import bacc

    f32 = mybir.dt.float32
    f16 = mybir.dt.float16
    bf16 = mybir.dt.bfloat16
    i8 = mybir.dt.int8
    AF = mybir.ActivationFunctionType

    nc = bacc.Bacc(
        "TRN2", target_bir_lowering=False, debug=False, num_devices=N_CORES
    )
    x_d = nc.dram_tensor("xt", [BPC, D, S], i8, kind="ExternalInput").ap()
    y_d = nc.dram_tensor("y", [BPC, D, S], i8, kind="ExternalInput").ap()
    o_d = nc.dram_tensor("out", [BPC, S, S], f16, kind="ExternalOutput").ap()

    # partition = bt*64+d; per (partition, pair) a contiguous 1KB DRAM run
    xv = x_d.rearrange("(c b2) d s -> (b2 d) c s", b2=2)
    yv = y_d.rearrange("(c b2) d s -> (b2 d) c s", b2=2)
    # out[b, 8p+r, t] <- ovn[b, p, r, t]: m-tile r covers rows {8p + r},
    # so one store's rows per partition are gsize*2KB contiguous in DRAM
    ovn = o_d.rearrange("b (p r) t -> b p r t", p=128, r=8)

    with tile.TileContext(nc) as tc, ExitStack() as ctx:
        const_pool = ctx.enter_context(tc.tile_pool(name="const", bufs=1))
        xin_pool = ctx.enter_context(tc.tile_pool(name="xin", bufs=1))
        yin_pool = ctx.enter_context(tc.tile_pool(name="yin", bufs=1))
        xbf_pool = ctx.enter_context(tc.tile_pool(name="xbf", bufs=3))
        ybf_pool = ctx.enter_context(tc.tile_pool(name="ybf", bufs=3))
        stage_pool = ctx.enter_context(tc.tile_pool(name="stage", bufs=12))
        mpsum_pool = ctx.enter_context(
            tc.tile_pool(name="mpsum", bufs=4, space="PSUM")
        )

        # (No HAM warmup: warmup matmuls themselves run cold/K=4 and
        # delay pair-0's mains ~1.4us past dequant readiness; the real
        # matmul stream earns the K=8 release just as fast.)
        x_sb = xin_pool.tile([128, NPAIRS, S], i8)
        y_sb = yin_pool.tile([128, NPAIRS, S], i8)

        # All loads issue up front: the whole 1.57MB flows during the
        # pre-store ramp while HBM is otherwise idle, instead of stealing
        # ~4us of store bandwidth mid-stream. Batched as 4 multi-run DMAs
        # (issue costs ~630ns each on sync; 12 singles would block the
        # first store issue by ~1.5us). Pairs 0-1 ride separately so
        # pair-0 dequant isn't gated on the full 1.57MB.
        nc.sync.dma_start(out=x_sb[:, 0:2, :], in_=xv[:, 0:2, :])
        nc.sync.dma_start(out=y_sb[:, 0:2, :], in_=yv[:, 0:2, :])
        nc.sync.dma_start(out=x_sb[:, 2:, :], in_=xv[:, 2:, :])
        nc.sync.dma_start(out=y_sb[:, 2:, :], in_=yv[:, 2:, :])

        # Prep (dequant) is software-pipelined two pairs ahead of the
        # matmul/store phase so pair boundaries don't stall the copies.
        preps = {}

        def prep(c, y_on_dve=False):
            xbf = xbf_pool.tile([128, S], bf16, tag="xbf")
            nc.scalar.activation(
                out=xbf[:], in_=x_sb[:, c, :], func=AF.Copy, bias=-az, scale=1.0
            )
            ybf = ybf_pool.tile([128, S], bf16, tag="ybf")
            if y_on_dve:
                # pair 0 only: x/y dequant concurrently on ACT/DVE so the
                # first matmul isn't gated on two serial ACT ops
                nc.vector.tensor_scalar_add(ybf[:], y_sb[:, c, :], -bz)
            else:
                nc.scalar.activation(
                    out=ybf[:], in_=y_sb[:, c, :], func=AF.Copy, bias=-bz, scale=1.0
                )
            # lhsT view: [128(bt,d), 8(r), 128(p)] with free stride 8
            preps[c] = (xbf.rearrange("q (p r) -> q r p", r=8), ybf)

        prep(0, y_on_dve=True)

        for c in range(NPAIRS):
            xtv, ybf = preps.pop(c)
            # gsize = r-tiles per store; small first groups for an early
            # first store, small last groups to shorten the drain tail
            gplan = (
                [2, 2, 4] if c == 0
                else ([4, 2, 1, 1] if c == NPAIRS - 1 else [4, 4])
            )
            g0 = 0
            for gi, gsize in enumerate(gplan):
                stages = []
                for bt in range(2):
                    stg = stage_pool.tile([128, gsize, S], f16, tag="stage")
                    stages.append(stg)
                for j in range(gsize):
                    m = g0 + j
                    pss = []
                    for bt in range(2):
                        ps = mpsum_pool.tile([128, S], f32, tag="mpsum")
                        pss.append(ps)
                    # e/o matmuls issued adjacently so the row-tiled PE
                    # overlaps their drain/fill
                    for nh in range(2):
                        for bt in range(2):
                            nc.tensor.matmul(
                                pss[bt][:, nh * 512 : (nh + 1) * 512],
                                xtv[bt * 64 : (bt + 1) * 64, m, :],
                                ybf[bt * 64 : (bt + 1) * 64, nh * 512 : (nh + 1) * 512],
                                start=True,
                                stop=True,
                                tile_position=(bt * 64, 0),
                            )
                    # pair 0: crosswise split so each stage fills via ACT
                    # and DVE in parallel (first stores ~1us earlier);
                    # steady state: the optimal ACT share of the 16
                    # copies/pair is fractional (~7.4: ACT owns the
                    # ~2.3us dequant at 1147ns/copy vs DVE's 1218ns), so
                    # alternate 7/8 by pair parity.
                    for bt in range(2):
                        idx = (g0 + j) * 2 + bt
                        act_set = (
                            (0, 2, 4, 7, 9, 11, 14)
                            if c % 2
                            else (0, 2, 4, 6, 9, 11, 13, 15)
                        )
                        on_act = (
                            idx in act_set if c else ((j + bt) % 2 == 0)
                        )
                        if on_act:
                            nc.scalar.activation(
                                out=stages[bt][:, j, :],
                                in_=pss[bt][:],
                                func=AF.Copy,
                                scale=al,
                            )
                        else:
                            nc.vector.tensor_scalar_mul(
                                stages[bt][:, j, :], pss[bt][:], al
                            )
                for bt in range(2):
                    nc.sync.dma_start(
                        out=ovn[2 * c + bt][:, g0 : g0 + gsize, :],
                        in_=stages[bt][:],
                    )
                # prep(1) waits until pair-0's first copies are queued so
                # its dequants don't delay the first store on ACT
                if c == 0 and gi == 0:
                    prep(1)
                g0 += gsize
            if c + 2 < NPAIRS:
                prep(c + 2)

    nc.compile()
    _cache[key] = nc
    return nc


def run_sharded(x, y, az, bz, al, trace=False, tmpdir=None):
    """Shard inputs over 8 cores, run, gather. Returns (out, BassKernelResults)."""
    from concourse.bass_utils import run_bass_kernel_spmd

    nc = _build(az, bz, al)
    xt = np.ascontiguousarray(x.transpose(0, 2, 1))  # host pre-transpose
    in_maps = [
        {
            "xt": xt[i * BPC : (i + 1) * BPC],
            "y": y[i * BPC : (i + 1) * BPC],
        }
        for i in range(N_CORES)
    ]
    res = run_bass_kernel_spmd(
        nc, in_maps, list(range(N_CORES)), trace=trace, tmpdir=tmpdir
    )
    out = np.concatenate(
        [r["out"].astype(np.float32) for r in res.results], axis=0
    )
    return out, res


def kernel(x, y, a_zp, b_zp, alpha):
    x = np.ascontiguousarray(np.asarray(x).astype(np.int8, copy=False))
    y = np.ascontiguousarray(np.asarray(y).astype(np.int8, copy=False))
    az = float(np.asarray(a_zp))
    bz = float(np.asarray(b_zp))
    al = float(np.asarray(alpha))
    out, _ = run_sharded(x, y, az, bz, al)
    return out


# revision 34
# speedup vs baseline: 1.0040x; 1.0040x over previous
"""Trainium2 Bass kernel: batched int8 dequant-BMM.

out[b] = (x[b].f32 - a_zp) @ (y[b].f32 - b_zp) * alpha
  x: [96, 1024, 64] int8, y: [96, 64, 1024] int8 -> out: [96, 1024, 1024] f32

Sharding: batch dim 96 -> 12 per core across 8 cores (pure data parallel).

The store stream saturates HBM (~358 GB/s/core), which trips the chip's
activity throttle: the PE is clock-gated to K=4/8 (~1.2 col/ns) for the
whole main phase, making PE column count the binding roofline. Hence:
  - x is pre-transposed on the HOST (numpy) so the kernel needs no PE
    transposes (saves 6144 PE columns/core + the DVE copy-backs + the
    identity constant; lhsT reads the dequanted x_T with a stride-8
    free-dim AP instead).
  - output is stored as fp16 and upcast on the host: the PSUM value is
    the exact integer result; alpha-scale + fp16 round gives max rel
    err 2^-11 ~= 4.9e-4, 40x inside the 2e-2 gate, and halves the
    store traffic.

Per-core pipeline (batch pair at a time; even batch on PE rows 0-63,
odd batch on rows 64-127 so the K=64 contractions row-tile the PE):
  DMA x_T,y int8 -> ACT dequant to bf16 (exact: integers < 256)
  -> PE matmul bf16 -> fp32 PSUM (exact: sums < 2^24)
  -> ACT/DVE copy PSUM->SBUF fused with *alpha, cast fp16 -> DMA out.

All DMA rides the two HWDGE rings. dma_start costs ~630ns on the
issuing engine, so the sync ring carries everything (loads interleaved
between store groups) and ACT issues none; the scalar ring is unused.
HAM warmup (~3.4us of dummy matmuls on memset data while PE waits on
the first loads) buys K=8/8 for the first ~2 pairs of real matmuls.
"""

import numpy as np

B, S, D = 96, 1024, 64
N_CORES = 8
BPC = B // N_CORES  # batches per core = 12
NPAIRS = BPC // 2

_cache = {}


def _build(az: float, bz: float, al: float):
    key = (az, bz, al)
    if key in _cache:
        return _cache[key]

    from contextlib import ExitStack

    import concourse.mybir as mybir
    import concourse.tile as tile
    from concourse import bacc

    f32 = mybir.dt.float32
    f16 = mybir.dt.float16
    bf16 = mybir.dt.bfloat16
    i8 = mybir.dt.int8
    AF = mybir.ActivationFunctionType

    nc = bacc.Bacc(
        "TRN2", target_bir_lowering=False, debug=False, num_devices=N_CORES
    )
    x_d = nc.dram_tensor("xt", [BPC, D, S], i8, kind="ExternalInput").ap()
    y_d = nc.dram_tensor("y", [BPC, D, S], i8, kind="ExternalInput").ap()
    o_d = nc.dram_tensor("out", [BPC, S, S], f16, kind="ExternalOutput").ap()

    # partition = bt*64+d; per (partition, pair) a contiguous 1KB DRAM run
    xv = x_d.rearrange("(c b2) d s -> (b2 d) c s", b2=2)
    yv = y_d.rearrange("(c b2) d s -> (b2 d) c s", b2=2)
    # out[b, 8p+r, t] <- ovn[b, p, r, t]: m-tile r covers rows {8p + r},
    # so one store's rows per partition are gsize*2KB contiguous in DRAM
    ovn = o_d.rearrange("b (p r) t -> b p r t", p=128, r=8)

    with tile.TileContext(nc) as tc, ExitStack() as ctx:
        const_pool = ctx.enter_context(tc.tile_pool(name="const", bufs=1))
        xin_pool = ctx.enter_context(tc.tile_pool(name="xin", bufs=1))
        yin_pool = ctx.enter_context(tc.tile_pool(name="yin", bufs=1))
        xbf_pool = ctx.enter_context(tc.tile_pool(name="xbf", bufs=3))
        ybf_pool = ctx.enter_context(tc.tile_pool(name="ybf", bufs=3))
        stage_pool = ctx.enter_context(tc.tile_pool(name="stage", bufs=12))
        mpsum_pool = ctx.enter_context(
            tc.tile_pool(name="mpsum", bufs=4, space="PSUM")
        )

        # (No HAM warmup: warmup matmuls themselves run cold/K=4 and
        # delay pair-0's mains ~1.4us past dequant readiness; the real
        # matmul stream earns the K=8 release just as fast.)
        x_sb = xin_pool.tile([128, NPAIRS, S], i8)
        y_sb = yin_pool.tile([128, NPAIRS, S], i8)

        # All loads issue up front: the whole 1.57MB flows during the
        # pre-store ramp while HBM is otherwise idle, instead of stealing
        # ~4us of store bandwidth mid-stream. Batched as 4 multi-run DMAs
        # (issue costs ~630ns each on sync; 12 singles would block the
        # first store issue by ~1.5us). Pairs 0-1 ride separately so
        # pair-0 dequant isn't gated on the full 1.57MB.
        nc.sync.dma_start(out=x_sb[:, 0:2, :], in_=xv[:, 0:2, :])
        nc.sync.dma_start(out=y_sb[:, 0:2, :], in_=yv[:, 0:2, :])
        nc.sync.dma_start(out=x_sb[:, 2:, :], in_=xv[:, 2:, :])
        nc.sync.dma_start(out=y_sb[:, 2:, :], in_=yv[:, 2:, :])

        # Prep (dequant) is software-pipelined two pairs ahead of the
        # matmul/store phase so pair boundaries don't stall the copies.
        preps = {}

        def prep(c, y_on_dve=False):
            xbf = xbf_pool.tile([128, S], bf16, tag="xbf")
            nc.scalar.activation(
                out=xbf[:], in_=x_sb[:, c, :], func=AF.Copy, bias=-az, scale=1.0
            )
            ybf = ybf_pool.tile([128, S], bf16, tag="ybf")
            if y_on_dve:
                # pair 0 only: x/y dequant concurrently on ACT/DVE so the
                # first matmul isn't gated on two serial ACT ops
                nc.vector.tensor_scalar_add(ybf[:], y_sb[:, c, :], -bz)
            else:
                nc.scalar.activation(
                    out=ybf[:], in_=y_sb[:, c, :], func=AF.Copy, bias=-bz, scale=1.0
                )
            # lhsT view: [128(bt,d), 8(r), 128(p)] with free stride 8
            preps[c] = (xbf.rearrange("q (p r) -> q r p", r=8), ybf)

        prep(0, y_on_dve=True)

        for c in range(NPAIRS):
            xtv, ybf = preps.pop(c)
            # gsize = r-tiles per store; small first groups for an early
            # first store, small last groups to shorten the drain tail
            gplan = (
                [2, 2, 4] if c == 0
                else ([4, 2, 2] if c == NPAIRS - 1 else [4, 4])
            )
            g0 = 0
            for gi, gsize in enumerate(gplan):
                stages = []
                for bt in range(2):
                    stg = stage_pool.tile([128, gsize, S], f16, tag="stage")
                    stages.append(stg)
                for j in range(gsize):
                    m = g0 + j
                    pss = []
                    for bt in range(2):
                        ps = mpsum_pool.tile([128, S], f32, tag="mpsum")
                        pss.append(ps)
                    # e/o matmuls issued adjacently so the row-tiled PE
                    # overlaps their drain/fill
                    for nh in range(2):
                        for bt in range(2):
                            nc.tensor.matmul(
                                pss[bt][:, nh * 512 : (nh + 1) * 512],
                                xtv[bt * 64 : (bt + 1) * 64, m, :],
                                ybf[bt * 64 : (bt + 1) * 64, nh * 512 : (nh + 1) * 512],
                                start=True,
                                stop=True,
                                tile_position=(bt * 64, 0),
                            )
                    # pair 0: crosswise split so each stage fills via ACT
                    # and DVE in parallel (first stores ~1us earlier);
                    # steady state: the optimal ACT share of the 16
                    # copies/pair is fractional (~7.4: ACT owns the
                    # ~2.3us dequant at 1147ns/copy vs DVE's 1218ns), so
                    # alternate 7/8 by pair parity.
                    for bt in range(2):
                        idx = (g0 + j) * 2 + bt
                        act_set = (
                            (0, 2, 4, 7, 9, 11, 14)
                            if c % 2
                            else (0, 2, 4, 6, 9, 11, 13, 15)
                        )
                        on_act = (
                            idx in act_set if c else ((j + bt) % 2 == 0)
                        )
                        if on_act:
                            nc.scalar.activation(
                                out=stages[bt][:, j, :],
                                in_=pss[bt][:],
                                func=AF.Copy,
                                scale=al,
                            )
                        else:
                            nc.vector.tensor_scalar_mul(
                                stages[bt][:, j, :], pss[bt][:], al
                            )
                for bt in range(2):
                    nc.sync.dma_start(
                        out=ovn[2 * c + bt][:, g0 : g0 + gsize, :],
                        in_=stages[bt][:],
                    )
                # prep(1) waits until pair-0's first copies are queued so
                # its dequants don't delay the first store on ACT
                if c == 0 and gi == 0:
                    prep(1)
                g0 += gsize
            if c + 2 < NPAIRS:
                prep(c + 2)

    nc.compile()
    _cache[key] = nc
    return nc


def run_sharded(x, y, az, bz, al, trace=False, tmpdir=None):
    """Shard inputs over 8 cores, run, gather. Returns (out, BassKernelResults)."""
    from concourse.bass_utils import run_bass_kernel_spmd

    nc = _build(az, bz, al)
    xt = np.ascontiguousarray(x.transpose(0, 2, 1))  # host pre-transpose
    in_maps = [
        {
            "xt": xt[i * BPC : (i + 1) * BPC],
            "y": y[i * BPC : (i + 1) * BPC],
        }
        for i in range(N_CORES)
    ]
    res = run_bass_kernel_spmd(
        nc, in_maps, list(range(N_CORES)), trace=trace, tmpdir=tmpdir
    )
    out = np.concatenate(
        [r["out"].astype(np.float32) for r in res.results], axis=0
    )
    return out, res


def kernel(x, y, a_zp, b_zp, alpha):
    x = np.ascontiguousarray(np.asarray(x).astype(np.int8, copy=False))
    y = np.ascontiguousarray(np.asarray(y).astype(np.int8, copy=False))
    az = float(np.asarray(a_zp))
    bz = float(np.asarray(b_zp))
    al = float(np.asarray(alpha))
    out, _ = run_sharded(x, y, az, bz, al)
    return out
